# revision 2
# baseline (speedup 1.0000x reference)
"""v3: DMA-xbar transposes for steady state; ACT runs gapless.

Per core (one kv head, its G=4 q heads, both batches): scores [k, q] via
KT^T@QT into 3 rotating psum bufs of W=1024; exp on ACT (the bottleneck,
~1038ns/group) into 4 rotating f16 PT sbuf bufs; O accumulates PT^T@V (ones
column = softmax denominator) into 2 opsum slots; DVE normalizes.

Prologue: K(b0) and Q(h0) tiles 0-7 load as f32 chunks on the SP HWDGE
queue and are PE-transposed (f32) into 24 unique psum strips (no strip
reuse -> no WAR chains), then DVE-copied to f16 KT/QT. Everything else
(Q(h0) tiles 8-15, 7 Q heads, K(b1)) loads via f32->f16 converting SWDGE
DMAs and is transposed per 128x128 tile by InstDmaTransposeAnt on the SP
queue, so the steady state never couples PE<->DVE. V loads are split so
O(0) isn't gated late; h6/h7 stores ride SP in quarter-head chunks to
shorten the tail. Junk matmuls on a zeroed tile warm the PE p-state
through the first DMA wait.
"""
import numpy as np
import concourse.bass as bass
from concourse import mybir
from contextlib import ExitStack

F32 = mybir.dt.float32
F16 = mybir.dt.float16
EXP = mybir.ActivationFunctionType.Exp
SCALE = float(1.0 / np.sqrt(128.0))

N_CORES = 8


def build_attention_nc(SEQ=2048, B=2, G=4):
    D = 128
    T = SEQ // 128          # 16 tiles along seq
    KG = 8                  # k-tiles per score group
    NKP = T // KG           # 2 k-phases per q-tile
    NQC = T                 # q-tiles per head
    H = B * G               # 8 (b, g) pairs per core
    W = KG * 128            # 1024 score cols per group
    NG = H * NQC * NKP      # 256 groups
    NPT = 4                 # PT ring depth

    nc = bass.Bass()
    q_ext = nc.declare_dram_parameter("query", [SEQ, B, G, D], F32, isOutput=False)
    k_ext = nc.declare_dram_parameter("key", [SEQ, B, D], F32, isOutput=False)
    v_ext = nc.declare_dram_parameter("value", [SEQ, B, D], F32, isOutput=False)
    o_ext = nc.declare_dram_parameter("out", [SEQ, B, G, D], F32, isOutput=True)

    # main (DMA-transposed) loads in first-use order
    main_loads = [("Q", 0, 1), ("Q", 0, 2), ("Q", 0, 3), ("K", 1, None),
                  ("Q", 1, 0), ("Q", 1, 1), ("Q", 1, 2), ("Q", 1, 3)]
    NM = len(main_loads)
    N_PE_TR = 24

    def head_mload(h):
        return h - 1 if h < G else h

    # ---------------- tensors ----------------
    ident = nc.alloc_sbuf_tensor("ident", [128, 128], F32)
    bias0 = nc.alloc_sbuf_tensor("bias0", [128, 1], F32)
    jz = nc.alloc_sbuf_tensor("jz", [128, 128], F16)
    qnK = nc.alloc_sbuf_tensor("qnK", [128, T * 128], F32)
    qnQ = nc.alloc_sbuf_tensor("qnQ", [128, 8 * 128], F32)
    qhi = nc.alloc_sbuf_tensor("qhi", [128, 8 * 128], F16)
    qm = [nc.alloc_sbuf_tensor(f"qm{i}", [128, T * 128], F16) for i in range(3)]
    KT = [nc.alloc_sbuf_tensor(f"KT{b}", [128, T * 128], F16) for b in range(B)]
    QT = [nc.alloc_sbuf_tensor(f"QT{h}", [128, T * 128], F16) for h in range(H)]
    VT = [nc.alloc_sbuf_tensor(f"VT{b}", [128, T * 132], F16) for b in range(B)]
    PT = [nc.alloc_sbuf_tensor(f"PT{s}", [128, W], F16) for s in range(NPT)]
    rsb = [nc.alloc_sbuf_tensor(f"rsb{s}", [128, 1], F32) for s in range(2)]
    OS = [nc.alloc_sbuf_tensor(f"OS{s}", [128, T * 128], F32) for s in range(2)]
    psum = nc.alloc_psum_tensor("psum", [128, 4096], F32)

    def spsum(s):
        return psum[:, s * W:(s + 1) * W]

    def opsum(buf):
        off = 3072 + buf * 512
        return psum[:, off:off + 129]

    STRIP_OFF = {20: 3201, 21: 3329, 22: 3840, 23: 3968}

    def strip(j):
        # 24 unique 128x128 f32 transpose strips (j<20 inside the future
        # score region, 20-23 tucked around the opsums)
        off = STRIP_OFF.get(j, 128 * j)
        return psum[:, off:off + 128]

    # prologue: five SP f32 chunk-loads, PE-transposed in arrival order
    #   ph0 j0-7:   K(b0) t0-7   ph1 j8-11: Q(h0) t0-3
    #   ph2 j12-15: K(b0) t8-11  ph3 j16-19: Q(h0) t4-7
    #   ph4 j20-23: K(b0) t12-15
    def tr_info(j):
        if j < 8:
            return qnK, j
        if j < 12:
            return qnQ, j - 8
        if j < 16:
            return qnK, 8 + (j - 12)
        if j < 20:
            return qnK, 12 + (j - 16)
        return qnQ, 4 + (j - 20)

    # copies: (psum src col, n tiles, dst is_q, dst t0, last tr j)
    COPIES = [(0, 8, 0, 0, 7), (1024, 4, 1, 0, 11), (1536, 4, 0, 8, 15),
              (2048, 4, 0, 12, 19), (3201, 2, 1, 4, 21), (3840, 2, 1, 6, 23)]
    N_CP = len(COPIES)

    # PE positions: trs j0-19, S(0), S(1), trs j20-23, then S(e)+O(e-2)
    pe_after_tr = {}
    pe_after_S = {}
    pe_after_O = {}
    pe = 0
    for j in range(20):
        pe += 1
        pe_after_tr[j] = pe
    pe += KG
    pe_after_S[0] = pe
    pe += KG
    pe_after_S[1] = pe
    for j in range(20, 24):
        pe += 1
        pe_after_tr[j] = pe
    for e in range(2, NG):
        pe += KG
        pe_after_S[e] = pe
        pe += KG
        pe_after_O[e - 2] = pe
    pe_after_O[NG - 2] = pe + KG
    pe_after_O[NG - 1] = pe + 2 * KG

    # DVE positions: prologue copies, then recip+mult per q-tile
    recips_done = {}
    mults_done = {}
    dve = N_CP
    for Qi in range(H * NQC):
        dve += 1
        recips_done[Qi] = dve
        dve += 1
        mults_done[Qi] = dve

    def eidx(e):
        kp = e % NKP
        Qi = e // NKP
        return Qi // NQC, Qi % NQC, kp, Qi

    with ExitStack() as ctx:
        sem_pe = ctx.enter_context(nc.semaphore("sem_pe"))
        sem_act = ctx.enter_context(nc.semaphore("sem_act"))
        sem_dve = ctx.enter_context(nc.semaphore("sem_dve"))
        sem_pool = ctx.enter_context(nc.semaphore("sem_pool"))
        sem_pl = [ctx.enter_context(nc.semaphore(f"sem_pl{i}"))
                  for i in range(5)]
        sem_qh = ctx.enter_context(nc.semaphore("sem_qh"))
        sem_lm = [ctx.enter_context(nc.semaphore(f"sem_lm{i}"))
                  for i in range(NM)]
        sem_tm = [ctx.enter_context(nc.semaphore(f"sem_tm{i}"))
                  for i in range(NM)]
        sem_tq0 = ctx.enter_context(nc.semaphore("sem_tq0"))
        sem_out = [ctx.enter_context(nc.semaphore(f"sem_out{h}"))
                   for h in range(H)]
        sem_v = [ctx.enter_context(nc.semaphore(f"sem_v{b}")) for b in range(B)]
        block = ctx.enter_context(nc.Block())

        def msrc(i):
            kind, b, g = main_loads[i]
            src = k_ext[:, b, :] if kind == "K" else q_ext[:, b, g, :]
            return src.rearrange("(t p) d -> p t d", p=128)

        def mdst(i):
            kind, b, g = main_loads[i]
            return KT[b] if kind == "K" else QT[b * G + g]

        @block.sync
        def _(sync):
            # prologue f32 chunk loads on the SP HWDGE queue
            chunks = [(k_ext[0:1024, 0, :], qnK, 0, 8),
                      (q_ext[0:512, 0, 0, :], qnQ, 0, 4),
                      (k_ext[1024:1536, 0, :], qnK, 8, 4),
                      (k_ext[1536:2048, 0, :], qnK, 12, 4),
                      (q_ext[512:1024, 0, 0, :], qnQ, 4, 4)]
            for i, (src, dst, t0, nt) in enumerate(chunks):
                nc.sync.dma_start(
                    out=dst[:, t0 * 128:(t0 + nt) * 128].rearrange(
                        "p (t d) -> p t d", d=128),
                    in_=src.rearrange("(t p) d -> p t d", p=128),
                ).then_inc(sem_pl[i], 16)
            # Q(h0) tiles 8-15 via DMA transpose (from Pool's converting load)
            nc.sync.wait_ge(sem_qh, 16)
            for t in range(8):
                nc.sync.dma_start_transpose(
                    out=QT[0][:, (8 + t) * 128:(9 + t) * 128],
                    in_=qhi[:, t * 128:(t + 1) * 128],
                ).then_inc(sem_tq0, 16)
            # DMA transposes for main loads
            for i in range(NM):
                nc.sync.wait_ge(sem_lm[i], 16)
                dst = mdst(i)
                src = qm[i % 3]
                for t in range(T):
                    nc.sync.dma_start_transpose(
                        out=dst[:, t * 128:(t + 1) * 128],
                        in_=src[:, t * 128:(t + 1) * 128],
                    ).then_inc(sem_tm[i], 16)
            # h6/h7 stores in quarter-head chunks (short tail)
            for h in range(H - 2, H):
                b, g = divmod(h, G)
                oh = o_ext[:, b, g, :].rearrange("(t p) d -> p t d", p=128)
                osh = OS[h % 2][:].rearrange("p (t d) -> p t d", d=128)
                for qtr in range(4):
                    nc.sync.wait_ge(sem_dve, mults_done[h * NQC + qtr * 4 + 3])
                    nc.sync.dma_start(
                        out=oh[:, qtr * 4:(qtr + 1) * 4, :],
                        in_=osh[:, qtr * 4:(qtr + 1) * 4, :],
                    ).then_inc(sem_out[h], 16)
            for h in range(H - 2):
                nc.sync.wait_ge(sem_out[h], 32)
            for h in range(H - 2, H):
                nc.sync.wait_ge(sem_out[h], 64)

        @block.gpsimd
        def _(gp):
            vt = [VT[b][:].rearrange("p (t c) -> p t c", c=132) for b in range(B)]
            nc.gpsimd.memset(jz[:], 0.0).then_inc(sem_pool)
            nc.gpsimd.memset(ident[:], 0.0).then_inc(sem_pool)
            nc.gpsimd.wait_ge(sem_pool, 2)
            nc.gpsimd.affine_select(
                out=ident[:], in_=ident[:],
                compare_op=mybir.AluOpType.not_equal, fill=1.0,
                base=0, pattern=[[-1, 128]], channel_multiplier=1,
            ).then_inc(sem_pool)
            nc.gpsimd.memset(bias0[:], 0.0).then_inc(sem_pool)
            nc.gpsimd.memset(vt[0][:, :, 128:129], 1.0).then_inc(sem_pool)
            nc.gpsimd.memset(vt[1][:, :, 128:129], 1.0).then_inc(sem_pool)
            # V halves early (O(0) gate), then Q(h0) hi (casting), V(b1)
            nc.gpsimd.dma_start(
                out=vt[0][:, 0:8, 0:128],
                in_=v_ext[0:1024, 0, :].rearrange("(t p) d -> p t d", p=128),
            ).then_inc(sem_v[0], 16)
            nc.gpsimd.dma_start(
                out=vt[0][:, 8:16, 0:128],
                in_=v_ext[1024:2048, 0, :].rearrange("(t p) d -> p t d", p=128),
            ).then_inc(sem_v[0], 16)
            nc.gpsimd.dma_start(
                out=qhi[:].rearrange("p (t d) -> p t d", d=128),
                in_=q_ext[1024:2048, 0, 0, :].rearrange("(t p) d -> p t d", p=128),
            ).then_inc(sem_qh, 16)
            nc.gpsimd.dma_start(
                out=vt[1][:, :, 0:128],
                in_=v_ext[:, 1, :].rearrange("(t p) d -> p t d", p=128),
            ).then_inc(sem_v[1], 16)
            # main loads (f32->f16 casting) into the qm ring
            for i in range(NM):
                if i >= 3:
                    nc.gpsimd.wait_ge(sem_tm[i - 3], 16 * T)
                nc.gpsimd.dma_start(
                    out=qm[i % 3][:].rearrange("p (t d) -> p t d", d=128),
                    in_=msrc(i),
                ).then_inc(sem_lm[i], 16)
            # output stores for h0-h5
            for h in range(H - 2):
                b, g = divmod(h, G)
                half = NQC // 2
                oh = o_ext[:, b, g, :].rearrange("(t p) d -> p t d", p=128)
                osh = OS[h % 2][:].rearrange("p (t d) -> p t d", d=128)
                nc.gpsimd.wait_ge(sem_dve, mults_done[h * NQC + half - 1])
                nc.gpsimd.dma_start(
                    out=oh[:, 0:half, :], in_=osh[:, 0:half, :],
                ).then_inc(sem_out[h], 16)
                nc.gpsimd.wait_ge(sem_dve, mults_done[h * NQC + NQC - 1])
                nc.gpsimd.dma_start(
                    out=oh[:, half:NQC, :], in_=osh[:, half:NQC, :],
                ).then_inc(sem_out[h], 16)

        @block.tensor
        def _(te):
            def emit_O(e):
                h, qc, kp, Qi = eidx(e)
                b = h // G
                buf = Qi % 2
                vt3 = VT[b][:].rearrange("p (t c) -> p t c", c=132)
                if kp == 0 and Qi >= 2:
                    nc.tensor.wait_ge(sem_dve, mults_done[Qi - 2])
                if qc == 0 and h % G == 0:
                    if kp == 0:
                        nc.tensor.wait_ge(sem_v[b], 16)
                        nc.tensor.wait_ge(sem_pool, 5 + b)
                    elif kp == 1 and b == 0:
                        nc.tensor.wait_ge(sem_v[0], 32)
                for ki in range(KG):
                    kt = kp * KG + ki
                    inst = nc.tensor.matmul(
                        opsum(buf)[:, 0:129],
                        PT[e % NPT][:, ki * 128:ki * 128 + 128],
                        vt3[:, kt, 0:129],
                        start=(kt == 0), stop=(kt == T - 1),
                        skip_group_check=True,
                    )
                    if ki == 0:
                        inst._wait_ge(sem_act, e + 1)
                    inst.then_inc(sem_pe)

            # p-state warm-up on a zeroed f16 tile (lands in a dead strip)
            nc.tensor.wait_ge(sem_pool, 1)
            junk_out = psum[:, 3457:3521]
            for w in range(40):
                nc.tensor.matmul(
                    junk_out[:, 0:64], jz[:, 0:128], jz[:, 0:64],
                    start=True, stop=True, skip_group_check=True)

            def emit_S(e):
                h, qc, kp, Qi = eidx(e)
                b = h // G
                s = e % 3
                if e == 0:
                    nc.tensor.wait_ge(sem_dve, 2)    # KT lo + QT t0-3
                elif e == 1:
                    nc.tensor.wait_ge(sem_dve, 4)    # KT hi
                if h == 0 and qc == 8 and kp == 0:
                    nc.tensor.wait_ge(sem_tq0, 8 * 16)
                if qc == 0 and kp == 0 and h >= 1:
                    nc.tensor.wait_ge(sem_tm[head_mload(h)], 16 * T)
                    if h == G:
                        nc.tensor.wait_ge(sem_tm[3], 16 * T)     # KT b1
                for ki in range(KG):
                    kt = kp * KG + ki
                    inst = nc.tensor.matmul(
                        spsum(s)[:, ki * 128:(ki + 1) * 128],
                        KT[b][:, kt * 128:(kt + 1) * 128],
                        QT[h][:, qc * 128:(qc + 1) * 128],
                        start=True, stop=True, skip_group_check=True,
                    )
                    if ki == 0 and e >= 3:
                        inst._wait_ge(sem_act, e - 2)
                    inst.then_inc(sem_pe)

            nc.tensor.wait_ge(sem_pool, 3)
            PL_GATE = {0: 0, 8: 1, 12: 2, 16: 3, 20: 4}
            for j in range(20):
                if j in PL_GATE:
                    nc.tensor.wait_ge(sem_pl[PL_GATE[j]], 16)
                src_qn, tile = tr_info(j)
                nc.tensor.transpose(
                    strip(j),
                    src_qn[:, tile * 128:(tile + 1) * 128],
                    ident[:],
                ).then_inc(sem_pe)
            emit_S(0)
            emit_S(1)
            for j in range(20, 24):
                if j in PL_GATE:
                    nc.tensor.wait_ge(sem_pl[PL_GATE[j]], 16)
                src_qn, tile = tr_info(j)
                nc.tensor.transpose(
                    strip(j),
                    src_qn[:, tile * 128:(tile + 1) * 128],
                    ident[:],
                ).then_inc(sem_pe)
            for e in range(2, NG):
                emit_S(e)
                emit_O(e - 2)
            emit_O(NG - 2)
            emit_O(NG - 1)

        @block.scalar
        def _(sc):
            nc.scalar.wait_ge(sem_pool, 4)
            for e in range(NG):
                nc.scalar.activation(
                    out=PT[e % NPT][:, 0:W], in_=spsum(e % 3),
                    func=EXP, bias=bias0[:, 0:1], scale=SCALE,
                )._wait_ge(sem_pe, pe_after_S[e]).then_inc(sem_act)

        @block.vector
        def _(ve):
            for c, (sc0, nt, is_q, t0, lastj) in enumerate(COPIES):
                nc.vector.wait_ge(sem_pe, pe_after_tr[lastj])
                dst_t = QT[0] if is_q else KT[0]
                nc.vector.tensor_copy(
                    dst_t[:, t0 * 128:(t0 + nt) * 128],
                    psum[:, sc0:sc0 + 128 * nt],
                ).then_inc(sem_dve)
            for Qi in range(H * NQC):
                h, qc = divmod(Qi, NQC)
                buf = Qi % 2
                nc.vector.wait_ge(sem_pe, pe_after_O[Qi * NKP + NKP - 1])
                if Qi >= 2:
                    nc.vector.wait_ge(sem_dve, mults_done[Qi - 2])
                nc.vector.reciprocal(
                    rsb[buf][:, 0:1], opsum(buf)[:, 128:129]
                ).then_inc(sem_dve)
                nc.vector.wait_ge(sem_dve, recips_done[Qi])
                if qc == 0 and h >= 2:
                    nc.vector.wait_ge(sem_out[h - 2], 32)
                nc.vector.tensor_scalar(
                    OS[h % 2][:, qc * 128:(qc + 1) * 128],
                    opsum(buf)[:, 0:128],
                    rsb[buf][:, 0:1],
                    None,
                    op0=mybir.AluOpType.mult,
                ).then_inc(sem_dve)

    return nc


_NC = None


def _get_nc():
    global _NC
    if _NC is None:
        _NC = build_attention_nc(2048, 2, 4)
    return _NC


def kernel(query, key, value):
    from concourse.bass_utils import run_bass_kernel_spmd

    query = np.ascontiguousarray(query, dtype=np.float32)
    key = np.ascontiguousarray(key, dtype=np.float32)
    value = np.ascontiguousarray(value, dtype=np.float32)
    G = query.shape[2] // key.shape[2]
    nc = _get_nc()
    in_maps = []
    for c in range(N_CORES):
        in_maps.append({
            "query": np.ascontiguousarray(query[:, :, c * G:(c + 1) * G, :]),
            "key": np.ascontiguousarray(key[:, :, c, :]),
            "value": np.ascontiguousarray(value[:, :, c, :]),
        })
    res = run_bass_kernel_spmd(nc, in_maps, list(range(N_CORES)))
    out = np.empty_like(query)
    for c in range(N_CORES):
        out[:, :, c * G:(c + 1) * G, :] = res.results[c]["out"]
    return out


# revision 3
# speedup vs baseline: 1.1527x; 1.1527x over previous
"""v3: DMA-xbar transposes for steady state; ACT runs gapless.

Per core (one kv head, its G=4 q heads, both batches): scores [k, q] via
KT^T@QT into 3 rotating psum bufs of W=1024; exp on ACT (the bottleneck,
~1038ns/group) into 4 rotating f16 PT sbuf bufs; O accumulates PT^T@V (ones
column = softmax denominator) into 2 opsum slots; DVE normalizes.

Prologue: K(b0) and Q(h0) tiles 0-7 load as f32 chunks on the SP HWDGE
queue and are PE-transposed (f32) into 24 unique psum strips (no strip
reuse -> no WAR chains), then DVE-copied to f16 KT/QT. Everything else
(Q(h0) tiles 8-15, 7 Q heads, K(b1)) loads via f32->f16 converting SWDGE
DMAs and is transposed per 128x128 tile by InstDmaTransposeAnt on the SP
queue, so the steady state never couples PE<->DVE. V loads are split so
O(0) isn't gated late; h6/h7 stores ride SP in quarter-head chunks to
shorten the tail. Junk matmuls on a zeroed tile warm the PE p-state
through the first DMA wait.
"""
import os
import numpy as np
import concourse.bass as bass
from concourse import mybir
from contextlib import ExitStack

F32 = mybir.dt.float32
F16 = mybir.dt.float16
I16 = mybir.dt.int16
EXP = mybir.ActivationFunctionType.Exp
SCALE = float(1.0 / np.sqrt(128.0))
# Schraudolph exp on DVE: bits(f16) = round(a*s + b); 2^frac approximated
# linearly, C trims the systematic ripple (tuned end-to-end).
XA = float(np.log2(np.e) * 1024.0 * SCALE)
XB = float(15.0 * 1024.0 - 60.0)

N_CORES = 8


def build_attention_nc(SEQ=2048, B=2, G=4):
    D = 128
    T = SEQ // 128          # 16 tiles along seq
    KG = 8                  # k-tiles per score group
    NKP = T // KG           # 2 k-phases per q-tile
    NQC = T                 # q-tiles per head
    H = B * G               # 8 (b, g) pairs per core
    W = KG * 128            # 1024 score cols per group
    NG = H * NQC * NKP      # 256 groups
    NPT = 4                 # PT ring depth

    nc = bass.Bass()
    q_ext = nc.declare_dram_parameter("query", [SEQ, B, G, D], F32, isOutput=False)
    k_ext = nc.declare_dram_parameter("key", [SEQ, B, D], F32, isOutput=False)
    v_ext = nc.declare_dram_parameter("value", [SEQ, B, D], F32, isOutput=False)
    o_ext = nc.declare_dram_parameter("out", [SEQ, B, G, D], F32, isOutput=True)

    # main (DMA-transposed) loads in first-use order
    main_loads = [("Q", 0, 1), ("Q", 0, 2), ("Q", 0, 3), ("K", 1, None),
                  ("Q", 1, 0), ("Q", 1, 1), ("Q", 1, 2), ("Q", 1, 3)]
    NM = len(main_loads)
    N_PE_TR = 24

    def head_mload(h):
        return h - 1 if h < G else h

    # ---------------- tensors ----------------
    ident = nc.alloc_sbuf_tensor("ident", [128, 128], F32)
    bias0 = nc.alloc_sbuf_tensor("bias0", [128, 1], F32)
    jz = nc.alloc_sbuf_tensor("jz", [128, 128], F16)
    qnK = nc.alloc_sbuf_tensor("qnK", [128, T * 128], F32)
    qnQ = nc.alloc_sbuf_tensor("qnQ", [128, 8 * 128], F32)
    qhi = nc.alloc_sbuf_tensor("qhi", [128, 8 * 128], F16)
    qm = [nc.alloc_sbuf_tensor(f"qm{i}", [128, T * 128], F16) for i in range(3)]
    KT = [nc.alloc_sbuf_tensor(f"KT{b}", [128, T * 128], F16) for b in range(B)]
    QT = [nc.alloc_sbuf_tensor(f"QT{h}", [128, T * 128], F16) for h in range(H)]
    VT = [nc.alloc_sbuf_tensor(f"VT{b}", [128, T * 132], F16) for b in range(B)]
    PT = [nc.alloc_sbuf_tensor(f"PT{s}", [128, W], F16) for s in range(NPT)]
    rsb = [nc.alloc_sbuf_tensor(f"rsb{s}", [128, 1], F32) for s in range(2)]
    xtmp = [nc.alloc_sbuf_tensor(f"xtmp{s}", [128, 256], F16) for s in range(2)]
    ptx = nc.alloc_sbuf_tensor("ptx", [128, 256], F16)
    asc = nc.alloc_sbuf_tensor("asc", [128, 1], F32)
    bsc = nc.alloc_sbuf_tensor("bsc", [128, 1], F32)
    OS = [nc.alloc_sbuf_tensor(f"OS{s}", [128, T * 128], F32) for s in range(2)]
    psum = nc.alloc_psum_tensor("psum", [128, 4096], F32)

    def spsum(s):
        return psum[:, s * W:(s + 1) * W]

    def opsum(buf):
        off = 3072 + buf * 512
        return psum[:, off:off + 129]

    STRIP_OFF = {20: 3201, 21: 3329, 22: 3840, 23: 3968}

    def strip(j):
        # 24 unique 128x128 f32 transpose strips (j<20 inside the future
        # score region, 20-23 tucked around the opsums)
        off = STRIP_OFF.get(j, 128 * j)
        return psum[:, off:off + 128]

    # prologue: five SP f32 chunk-loads, PE-transposed in arrival order
    #   ph0 j0-7:   K(b0) t0-7   ph1 j8-11: Q(h0) t0-3
    #   ph2 j12-15: K(b0) t8-11  ph3 j16-19: Q(h0) t4-7
    #   ph4 j20-23: K(b0) t12-15
    def tr_info(j):
        if j < 8:
            return qnK, j
        if j < 12:
            return qnQ, j - 8
        if j < 16:
            return qnK, 8 + (j - 12)
        if j < 20:
            return qnK, 12 + (j - 16)
        return qnQ, 4 + (j - 20)

    # copies: (psum src col, n tiles, dst is_q, dst t0, last tr j)
    COPIES = [(0, 8, 0, 0, 7), (1024, 4, 1, 0, 11), (1536, 4, 0, 8, 15),
              (2048, 4, 0, 12, 19), (3201, 2, 1, 4, 21), (3840, 2, 1, 6, 23)]
    N_CP = len(COPIES)

    # exp column split: ACT exps cols [0:XC) of every group, DVE approximates
    # the tail [XC:W) with the Schraudolph bit-hack (tensor_scalar + convert).
    XW = 256            # DVE-approximated tail: ki 0,1
    XC = W - XW

    # PE emission walk: trs j0-19, S(0), S(1), trs j20-23, then S(e), O(e-2)
    pe_after_tr = {}
    pe_after_S = {}
    pe_after_O = {}
    pe = 0
    for j in range(20):
        pe += 1
        pe_after_tr[j] = pe
    pe += KG
    pe_after_S[0] = pe
    pe += KG
    pe_after_S[1] = pe
    for j in range(20, 24):
        pe += 1
        pe_after_tr[j] = pe
    for e in range(2, NG):
        pe += KG
        pe_after_S[e] = pe
        pe += KG
        pe_after_O[e - 2] = pe
    pe_after_O[NG - 2] = pe + KG
    pe_after_O[NG - 1] = pe + 2 * KG

    # DVE program walk: prologue copies, then per block w the exp-tail ops,
    # then norms due one block after their last O
    dve_ops = []
    for w in range(NG + 6):
        if w < NG and XW:
            dve_ops.append(("xp1", w))
        if 1 <= w <= NG and XW:
            dve_ops.append(("xp2", w - 1))   # pipelined: xp1(w) hides the RAW
        for Qi in range(H * NQC):
            if 2 * Qi + 4 == w:
                dve_ops.append(("recip", Qi))
                dve_ops.append(("mult", Qi))
    xp_pos = {}
    recips_done = {}
    mults_done = {}
    dve = N_CP
    for op, arg in dve_ops:
        dve += 1
        if op in ("xp1", "xp2"):
            xp_pos[(op, arg)] = dve
        elif op == "recip":
            recips_done[arg] = dve
        else:
            mults_done[arg] = dve

    def eidx(e):
        kp = e % NKP
        Qi = e // NKP
        return Qi // NQC, Qi % NQC, kp, Qi

    with ExitStack() as ctx:
        sem_pe = ctx.enter_context(nc.semaphore("sem_pe"))
        sem_act = ctx.enter_context(nc.semaphore("sem_act"))
        sem_dve = ctx.enter_context(nc.semaphore("sem_dve"))
        sem_pool = ctx.enter_context(nc.semaphore("sem_pool"))
        sem_pl = [ctx.enter_context(nc.semaphore(f"sem_pl{i}"))
                  for i in range(5)]
        sem_qh = ctx.enter_context(nc.semaphore("sem_qh"))
        sem_lm = [ctx.enter_context(nc.semaphore(f"sem_lm{i}"))
                  for i in range(NM)]
        sem_tm = [ctx.enter_context(nc.semaphore(f"sem_tm{i}"))
                  for i in range(NM)]
        sem_tq0 = ctx.enter_context(nc.semaphore("sem_tq0"))
        sem_out = [ctx.enter_context(nc.semaphore(f"sem_out{h}"))
                   for h in range(H)]
        sem_v = [ctx.enter_context(nc.semaphore(f"sem_v{b}")) for b in range(B)]
        sem_vh = ctx.enter_context(nc.semaphore("sem_vh"))
        block = ctx.enter_context(nc.Block())

        def msrc(i):
            kind, b, g = main_loads[i]
            src = k_ext[:, b, :] if kind == "K" else q_ext[:, b, g, :]
            return src.rearrange("(t p) d -> p t d", p=128)

        def mdst(i):
            kind, b, g = main_loads[i]
            return KT[b] if kind == "K" else QT[b * G + g]

        @block.sync
        def _(sync):
            # prologue f32 chunk loads on the SP HWDGE queue
            chunks = [(k_ext[0:1024, 0, :], qnK, 0, 8),
                      (q_ext[0:512, 0, 0, :], qnQ, 0, 4),
                      (k_ext[1024:1536, 0, :], qnK, 8, 4),
                      (k_ext[1536:2048, 0, :], qnK, 12, 4),
                      (q_ext[512:1024, 0, 0, :], qnQ, 4, 4)]
            for i, (src, dst, t0, nt) in enumerate(chunks):
                nc.sync.dma_start(
                    out=dst[:, t0 * 128:(t0 + nt) * 128].rearrange(
                        "p (t d) -> p t d", d=128),
                    in_=src.rearrange("(t p) d -> p t d", p=128),
                ).then_inc(sem_pl[i], 16)
            # Q(h0) tiles 8-15 via DMA transpose (from Pool's converting load)
            nc.sync.wait_ge(sem_qh, 16)
            for t in range(8):
                nc.sync.dma_start_transpose(
                    out=QT[0][:, (8 + t) * 128:(9 + t) * 128],
                    in_=qhi[:, t * 128:(t + 1) * 128],
                ).then_inc(sem_tq0, 16)
            # DMA transposes for main loads
            for i in range(NM):
                nc.sync.wait_ge(sem_lm[i], 16)
                dst = mdst(i)
                src = qm[i % 3]
                for t in range(T):
                    nc.sync.dma_start_transpose(
                        out=dst[:, t * 128:(t + 1) * 128],
                        in_=src[:, t * 128:(t + 1) * 128],
                    ).then_inc(sem_tm[i], 16)
            # h6/h7 stores in quarter-head chunks (short tail)
            for h in range(H - 2, H):
                b, g = divmod(h, G)
                oh = o_ext[:, b, g, :].rearrange("(t p) d -> p t d", p=128)
                osh = OS[h % 2][:].rearrange("p (t d) -> p t d", d=128)
                for qtr in range(4):
                    nc.sync.wait_ge(sem_dve, mults_done[h * NQC + qtr * 4 + 3])
                    nc.sync.dma_start(
                        out=oh[:, qtr * 4:(qtr + 1) * 4, :],
                        in_=osh[:, qtr * 4:(qtr + 1) * 4, :],
                    ).then_inc(sem_out[h], 16)
            for h in range(H - 2):
                nc.sync.wait_ge(sem_out[h], 32)
            for h in range(H - 2, H):
                nc.sync.wait_ge(sem_out[h], 64)

        @block.gpsimd
        def _(gp):
            vt = [VT[b][:].rearrange("p (t c) -> p t c", c=132) for b in range(B)]
            nc.gpsimd.memset(asc[:], XA)
            nc.gpsimd.memset(bsc[:], XB)
            nc.gpsimd.memset(jz[:], 0.0).then_inc(sem_pool)
            nc.gpsimd.memset(ident[:], 0.0).then_inc(sem_pool)
            nc.gpsimd.wait_ge(sem_pool, 2)
            nc.gpsimd.affine_select(
                out=ident[:], in_=ident[:],
                compare_op=mybir.AluOpType.not_equal, fill=1.0,
                base=0, pattern=[[-1, 128]], channel_multiplier=1,
            ).then_inc(sem_pool)
            nc.gpsimd.memset(bias0[:], 0.0).then_inc(sem_pool)
            nc.gpsimd.memset(vt[0][:, :, 128:129], 1.0).then_inc(sem_pool)
            nc.gpsimd.memset(vt[1][:, :, 128:129], 1.0).then_inc(sem_pool)
            # V halves early (O(0) gate), then Q(h0) hi (casting), V(b1)
            nc.gpsimd.dma_start(
                out=vt[0][:, 0:8, 0:128],
                in_=v_ext[0:1024, 0, :].rearrange("(t p) d -> p t d", p=128),
            ).then_inc(sem_v[0], 16)
            nc.gpsimd.dma_start(
                out=vt[0][:, 8:16, 0:128],
                in_=v_ext[1024:2048, 0, :].rearrange("(t p) d -> p t d", p=128),
            ).then_inc(sem_vh, 16)
            nc.gpsimd.dma_start(
                out=qhi[:].rearrange("p (t d) -> p t d", d=128),
                in_=q_ext[1024:2048, 0, 0, :].rearrange("(t p) d -> p t d", p=128),
            ).then_inc(sem_qh, 16)
            nc.gpsimd.dma_start(
                out=vt[1][:, :, 0:128],
                in_=v_ext[:, 1, :].rearrange("(t p) d -> p t d", p=128),
            ).then_inc(sem_v[1], 16)
            # main loads (f32->f16 casting) into the qm ring, interleaved
            # with h0-h5 half-head stores in due order (a store after all
            # mains would arrive ~30us late and stall the OS ring)
            def emit_main(i):
                if i >= 3:
                    nc.gpsimd.wait_ge(sem_tm[i - 3], 16 * T)
                nc.gpsimd.dma_start(
                    out=qm[i % 3][:].rearrange("p (t d) -> p t d", d=128),
                    in_=msrc(i),
                ).then_inc(sem_lm[i], 16)

            def emit_store(h, half_i):
                b, g = divmod(h, G)
                half = NQC // 2
                oh = o_ext[:, b, g, :].rearrange("(t p) d -> p t d", p=128)
                osh = OS[h % 2][:].rearrange("p (t d) -> p t d", d=128)
                t0 = half_i * half
                nc.gpsimd.wait_ge(sem_dve, mults_done[h * NQC + t0 + half - 1])
                nc.gpsimd.dma_start(
                    out=oh[:, t0:t0 + half, :], in_=osh[:, t0:t0 + half, :],
                ).then_inc(sem_out[h], 16)

            sched = [("m", 0), ("m", 1), ("m", 2), ("st", 0, 0), ("m", 3),
                     ("st", 0, 1), ("m", 4), ("st", 1, 0), ("m", 5), ("m", 6),
                     ("st", 1, 1), ("m", 7), ("st", 2, 0), ("st", 2, 1),
                     ("st", 3, 0), ("st", 3, 1), ("st", 4, 0), ("st", 4, 1),
                     ("st", 5, 0), ("st", 5, 1)]
            for item in sched:
                if item[0] == "m":
                    emit_main(item[1])
                else:
                    emit_store(item[1], item[2])

        @block.tensor
        def _(te):
            def emit_O(e):
                h, qc, kp, Qi = eidx(e)
                b = h // G
                buf = Qi % 2
                vt3 = VT[b][:].rearrange("p (t c) -> p t c", c=132)
                if kp == 0 and Qi >= 2:
                    nc.tensor.wait_ge(sem_dve, mults_done[Qi - 2])
                if qc == 0 and h % G == 0:
                    if kp == 0:
                        nc.tensor.wait_ge(sem_v[b], 16)
                        nc.tensor.wait_ge(sem_pool, 5 + b)
                    elif kp == 1 and b == 0:
                        nc.tensor.wait_ge(sem_vh, 16)
                if XW:
                    nc.tensor.wait_ge(sem_dve, xp_pos[("xp2", e)])
                for ki in range(KG):
                    kt = kp * KG + ki
                    inst = nc.tensor.matmul(
                        opsum(buf)[:, 0:129],
                        PT[e % NPT][:, ki * 128:ki * 128 + 128],
                        vt3[:, kt, 0:129],
                        start=(kt == 0), stop=(kt == T - 1),
                        skip_group_check=True,
                    )
                    if ki == 0:
                        inst._wait_ge(sem_act, e + 1)
                    inst.then_inc(sem_pe)

            # p-state warm-up on a zeroed f16 tile (lands in a dead strip)
            nc.tensor.wait_ge(sem_pool, 1)
            junk_out = psum[:, 3457:3521]
            for w in range(40):
                nc.tensor.matmul(
                    junk_out[:, 0:64], jz[:, 0:128], jz[:, 0:64],
                    start=True, stop=True, skip_group_check=True)

            def emit_S(e):
                h, qc, kp, Qi = eidx(e)
                b = h // G
                s = e % 3
                if e == 0:
                    nc.tensor.wait_ge(sem_dve, 2)    # KT lo + QT t0-3
                elif e == 1:
                    nc.tensor.wait_ge(sem_dve, 4)    # KT hi
                if h == 0 and qc == 8 and kp == 0:
                    nc.tensor.wait_ge(sem_tq0, 8 * 16)
                if qc == 0 and kp == 0 and h >= 1:
                    nc.tensor.wait_ge(sem_tm[head_mload(h)], 16 * T)
                    if h == G:
                        nc.tensor.wait_ge(sem_tm[3], 16 * T)     # KT b1
                if e >= 3 and XW:
                    nc.tensor.wait_ge(sem_dve, xp_pos[("xp2", e - 3)])
                for ki in range(KG):
                    kt = kp * KG + ki
                    inst = nc.tensor.matmul(
                        spsum(s)[:, ki * 128:(ki + 1) * 128],
                        KT[b][:, kt * 128:(kt + 1) * 128],
                        QT[h][:, qc * 128:(qc + 1) * 128],
                        start=True, stop=True, skip_group_check=True,
                    )
                    if ki == 0 and e >= 3:
                        inst._wait_ge(sem_act, e - 2)
                    inst.then_inc(sem_pe)

            nc.tensor.wait_ge(sem_pool, 3)
            PL_GATE = {0: 0, 8: 1, 12: 2, 16: 3, 20: 4}
            for j in range(20):
                if j in PL_GATE:
                    nc.tensor.wait_ge(sem_pl[PL_GATE[j]], 16)
                src_qn, tile = tr_info(j)
                nc.tensor.transpose(
                    strip(j),
                    src_qn[:, tile * 128:(tile + 1) * 128],
                    ident[:],
                ).then_inc(sem_pe)
            emit_S(0)
            emit_S(1)
            for j in range(20, 24):
                if j in PL_GATE:
                    nc.tensor.wait_ge(sem_pl[PL_GATE[j]], 16)
                src_qn, tile = tr_info(j)
                nc.tensor.transpose(
                    strip(j),
                    src_qn[:, tile * 128:(tile + 1) * 128],
                    ident[:],
                ).then_inc(sem_pe)
            for e in range(2, NG):
                emit_S(e)
                emit_O(e - 2)
            emit_O(NG - 2)
            emit_O(NG - 1)

        @block.scalar
        def _(sc):
            nc.scalar.wait_ge(sem_pool, 4)
            for e in range(NG):
                nc.scalar.wait_ge(sem_dve, xp_pos[("xp1", e)])
                nc.scalar.activation(
                    out=PT[e % NPT][:, XW:W], in_=spsum(e % 3)[:, XW:W],
                    func=EXP, bias=bias0[:, 0:1], scale=SCALE,
                )._wait_ge(sem_pe, pe_after_S[e]).then_inc(sem_act)

        @block.vector
        def _(ve):
            for c, (sc0, nt, is_q, t0, lastj) in enumerate(COPIES):
                nc.vector.wait_ge(sem_pe, pe_after_tr[lastj])
                dst_t = QT[0] if is_q else KT[0]
                nc.vector.tensor_copy(
                    dst_t[:, t0 * 128:(t0 + nt) * 128],
                    psum[:, sc0:sc0 + 128 * nt],
                ).then_inc(sem_dve)
            for op, arg in dve_ops:
                if op == "xp1":
                    e = arg
                    nc.vector.wait_ge(sem_pe, pe_after_S[e] - (KG - 2))
                    if e >= 2:
                        # xtmp ring WAR: previous reader of xtmp[e%2]
                        nc.vector.wait_ge(sem_dve, xp_pos[("xp2", e - 2)])
                    nc.vector.tensor_copy(
                        xtmp[e % 2][:, 0:XW],
                        spsum(e % 3)[:, 0:XW],
                    ).then_inc(sem_dve)
                elif op == "xp2":
                    e = arg
                    nc.vector.wait_ge(sem_dve, xp_pos[("xp1", e)])
                    nc.vector.tensor_scalar(
                        PT[e % NPT][:, 0:XW].bitcast(I16),
                        xtmp[e % 2][:, 0:XW], XA, XB,
                        op0=mybir.AluOpType.mult, op1=mybir.AluOpType.add,
                    ).then_inc(sem_dve)
                elif op == "recip":
                    Qi = arg
                    buf = Qi % 2
                    nc.vector.wait_ge(sem_pe, pe_after_O[Qi * NKP + NKP - 1])
                    if Qi >= 2:
                        nc.vector.wait_ge(sem_dve, mults_done[Qi - 2])
                    nc.vector.reciprocal(
                        rsb[buf][:, 0:1], opsum(buf)[:, 128:129]
                    ).then_inc(sem_dve)
                else:
                    Qi = arg
                    h, qc = divmod(Qi, NQC)
                    buf = Qi % 2
                    nc.vector.wait_ge(sem_dve, recips_done[Qi])
                    if qc == 0 and h >= 2:
                        nc.vector.wait_ge(sem_out[h - 2], 32)
                    nc.vector.tensor_scalar(
                        OS[h % 2][:, qc * 128:(qc + 1) * 128],
                        opsum(buf)[:, 0:128],
                        rsb[buf][:, 0:1],
                        None,
                        op0=mybir.AluOpType.mult,
                    ).then_inc(sem_dve)

    return nc


_NC = None


def _get_nc():
    global _NC
    if _NC is None:
        _NC = build_attention_nc(2048, 2, 4)
    return _NC


def kernel(query, key, value):
    from concourse.bass_utils import run_bass_kernel_spmd

    query = np.ascontiguousarray(query, dtype=np.float32)
    key = np.ascontiguousarray(key, dtype=np.float32)
    value = np.ascontiguousarray(value, dtype=np.float32)
    G = query.shape[2] // key.shape[2]
    nc = _get_nc()
    in_maps = []
    for c in range(N_CORES):
        in_maps.append({
            "query": np.ascontiguousarray(query[:, :, c * G:(c + 1) * G, :]),
            "key": np.ascontiguousarray(key[:, :, c, :]),
            "value": np.ascontiguousarray(value[:, :, c, :]),
        })
    res = run_bass_kernel_spmd(nc, in_maps, list(range(N_CORES)))
    out = np.empty_like(query)
    for c in range(N_CORES):
        out[:, :, c * G:(c + 1) * G, :] = res.results[c]["out"]
    return out
